# revision 19
# baseline (speedup 1.0000x reference)
"""Causal attention layer on 8 TRN2 NeuronCores, data-parallel over batch.

Per-core problem (batch element n = core id):
    q = query @ Wq.T ; k = key @ Wk.T              (f32r matmuls)
    scores[s,t] = q[s]·k[t]  for t <= s            (f32r)
    attn = softmax(32 * scores)  (the +1 additive mask cancels in softmax;
                                  -inf masking == skipping t > s)
    ctx[s,i] = sum_t attn[s,t] * value[t,i]        (bf16)
    out[s,:] = (ctx @ Wv.T) / rowsum               (f32r, normalization folded)

Layouts: qT/kT are built as [D, S] via PE transposes of the inputs and
weights so every matmul contracts along partitions without DMA transposes.
The strip loop is a 3-stage software pipeline (A: scores+max, B1: exp+AV,
B2: ctx-transpose+V-proj) so the PE always has matmuls queued while the
serial softmax chain runs on DVE/ACT.
"""
import numpy as np
from contextlib import ExitStack

import concourse.bass as bass
import concourse.tile as tile
from concourse import bacc, mybir
from concourse.bass_utils import run_bass_kernel_spmd
from concourse.masks import make_identity

F32 = mybir.dt.float32
F32R = mybir.dt.float32r
BF16 = mybir.dt.bfloat16

N, S, T, D = 8, 2048, 2048, 1024
P = 128
NSTRIP = S // P          # 16 query strips
TCH = 512                # t-chunk for score matmuls
CW = 512                 # projection chunk width
OC = D // P              # 8 chunks of the projection/feature dim
SCALE = float(np.sqrt(np.float32(D)))  # 32.0
NEG = -1.0e30

# dtype knobs (QK path needs >= f32r precision; see noise_sim.py)
QK_DT = F32R             # q/k projections + scores matmuls
AV_DT = BF16             # attn weights + value contraction
VP_DT = F32R             # final (attn@value) @ Wv.T projection

PHASE_MARKS = []


def _mark(nc, label):
    PHASE_MARKS.append((label, nc.next_id()))


def _mm(nc, out, lhsT, rhs, dt, **kw):
    nc.tensor.matmul(out, lhsT.bitcast(dt), rhs.bitcast(dt), **kw)


def build_nc():
    PHASE_MARKS.clear()
    nc = bacc.Bacc("TRN2", target_bir_lowering=False, debug=False,
                   enable_asserts=False)
    _dma_rr = [0]

    def dma(out, in_):
        # alternate between the two HWDGE queues (sync/SP and scalar/ACT)
        _dma_rr[0] ^= 1
        eng = nc.sync if _dma_rr[0] else nc.scalar
        return eng.dma_start(out, in_)
    q_d = nc.dram_tensor("query", [S, D], F32, kind="ExternalInput")
    k_d = nc.dram_tensor("key", [T, D], F32, kind="ExternalInput")
    v_d = nc.dram_tensor("value", [T, D], F32, kind="ExternalInput")
    wq_d = nc.dram_tensor("Wq", [D, D], F32, kind="ExternalInput")
    wk_d = nc.dram_tensor("Wk", [D, D], F32, kind="ExternalInput")
    wv_d = nc.dram_tensor("Wv", [D, D], F32, kind="ExternalInput")
    out_d = nc.dram_tensor("out", [S, D], F32, kind="ExternalOutput")

    with tile.TileContext(nc) as tc, ExitStack() as ctx:
        const = ctx.enter_context(tc.tile_pool(name="const", bufs=1))
        wt_pool = ctx.enter_context(tc.tile_pool(name="wt", bufs=1))
        kt_pool = ctx.enter_context(tc.tile_pool(name="kt", bufs=1))
        val_pool = ctx.enter_context(tc.tile_pool(name="val", bufs=1))
        stage = ctx.enter_context(tc.tile_pool(name="stage", bufs=3))
        int_pool = ctx.enter_context(tc.tile_pool(name="inT", bufs=1))
        qts_pool = ctx.enter_context(tc.tile_pool(name="qts", bufs=2))
        sc_pool = ctx.enter_context(tc.tile_pool(name="scores", bufs=2))
        exp_pool = ctx.enter_context(tc.tile_pool(name="exp", bufs=2))
        at_pool = ctx.enter_context(tc.tile_pool(name="attnT", bufs=1))
        ctxn_pool = ctx.enter_context(tc.tile_pool(name="ctxn", bufs=2))
        ctx_pool = ctx.enter_context(tc.tile_pool(name="ctxsb", bufs=1))
        ob_pool = ctx.enter_context(tc.tile_pool(name="outb", bufs=1))
        st_pool = ctx.enter_context(tc.tile_pool(name="stats", bufs=24))
        mm_ps = ctx.enter_context(tc.tile_pool(name="mmps", bufs=4, space="PSUM"))
        ctx_ps = ctx.enter_context(tc.tile_pool(name="ctxps", bufs=2, space="PSUM"))

        ident = const.tile([P, P], F32)
        make_identity(nc, ident)
        ident_bf = const.tile([P, P], BF16)
        nc.vector.tensor_copy(ident_bf[:], ident[:])

        # Additive diag masks for the 4 possible strip positions inside a
        # 512-wide t-chunk: mask[j][x, y] = 0 if y <= j*128 + x else -1e30
        # (bf16 is exact for both values)
        diagmask = const.tile([P, 4, TCH], BF16)
        nc.gpsimd.memset(diagmask[:], 0.0)
        for j in range(4):
            nc.gpsimd.affine_select(
                out=diagmask[:, j, :], in_=diagmask[:, j, :],
                compare_op=mybir.AluOpType.is_ge, fill=NEG,
                base=j * P, channel_multiplier=1, pattern=[[-1, TCH]])

        def load_wT(w_dram, dt):
            """[D,D] weight [o,i] -> SBUF [128(i_loc), OC(i_chunk), D(o)]."""
            wt = wt_pool.tile([P, OC, D], dt, name="wT", tag="wT")
            for r in range(OC):              # o-strip
                stg = stage.tile([P, D], F32, name="stage", tag="stage")
                dma(stg[:], w_dram.ap()[r * P:(r + 1) * P, :])
                for g in range(0, OC, 4):
                    ps = mm_ps.tile([P, TCH], F32, name="tps", tag="mmps")
                    for c in range(4):
                        nc.tensor.transpose(
                            ps[:, c * P:(c + 1) * P],
                            stg[:, (g + c) * P:(g + c + 1) * P], ident[:])
                    nc.vector.tensor_copy(
                        wt[:, g:g + 4, r * P:(r + 1) * P],
                        ps.rearrange("p (c s) -> p c s", c=4))
            return wt

        def in_transpose_chunk(x_dram, c0):
            """x[c0:c0+CW, :D] -> xT tile [128(i_loc), OC, CW]."""
            xt = int_pool.tile([P, OC, CW], QK_DT, name="inT", tag="inT")
            for sl in range(CW // P):
                stg = stage.tile([P, D], F32, name="stage", tag="stage")
                dma(
                    stg[:], x_dram.ap()[c0 + sl * P:c0 + (sl + 1) * P, :])
                for g in range(0, OC, 4):
                    ps = mm_ps.tile([P, TCH], F32, name="tps", tag="mmps")
                    for c in range(4):
                        nc.tensor.transpose(
                            ps[:, c * P:(c + 1) * P],
                            stg[:, (g + c) * P:(g + c + 1) * P], ident[:])
                    nc.vector.tensor_copy(
                        xt[:, g:g + 4, sl * P:(sl + 1) * P],
                        ps.rearrange("p (c s) -> p c s", c=4))
            return xt

        # ---- phase G: gT[j, i] = (Wq^T @ Wk)^T = Wk^T @ Wq ----
        # Both weights are consumed in their NATURAL [o, .] layout, so no
        # weight transposes are needed; Wq's projection of the query side
        # is folded into the key side via M = G @ key^T below.
        _mark(nc, 'phaseG')
        wqwk = kt_pool.tile([P, 16, D], QK_DT, name="wqwk", tag="kt")
        for r in range(OC):
            stg = stage.tile([P, D], F32, name="stage", tag="stage")
            dma(stg[:], wq_d.ap()[r * P:(r + 1) * P, :])
            nc.gpsimd.tensor_copy(wqwk[:, r, :], stg[:])
            stg2 = stage.tile([P, D], F32, name="stage", tag="stage")
            dma(stg2[:], wk_d.ap()[r * P:(r + 1) * P, :])
            nc.gpsimd.tensor_copy(wqwk[:, OC + r, :], stg2[:])
        kT_chunks = [in_transpose_chunk(k_d, 0)]
        gT = wt_pool.tile([P, OC, D], QK_DT, name="wT", tag="wT")
        # o-strip-streamed accumulation, 4 jc-accumulators at a time in the
        # (preamble-idle) ctx_ps banks; matmuls start as soon as the first
        # Wq/Wk strips land and the group copies overlap the next group
        for ih in range(2):
            for jcg in range(2):
                acc_b = ctx_ps.tile([P, 2, TCH], F32, name="ctxps",
                                    tag="ctxps")
                acc_c = ctx_ps.tile([P, 2, TCH], F32, name="ctxps",
                                    tag="ctxps")
                accs = [acc_b[:, 0, :], acc_b[:, 1, :],
                        acc_c[:, 0, :], acc_c[:, 1, :]]
                for oc in range(OC):
                    for j in range(4):
                        jc = jcg * 4 + j
                        _mm(nc, accs[j],
                            wqwk[:, OC + oc, jc * P:(jc + 1) * P],
                            wqwk[:, oc, ih * TCH:(ih + 1) * TCH], QK_DT,
                            start=(oc == 0), stop=(oc == OC - 1))
                for j in range(4):
                    jc = jcg * 4 + j
                    nc.vector.tensor_copy(
                        gT[:, jc, ih * TCH:(ih + 1) * TCH], accs[j])

        # ---- phase M: M = G @ key^T -> resident SBUF [128, OC, T] ----
        # (reuses the wqwk slot; M plays the role kT did: scores = query @ M)
        _mark(nc, 'phaseM')
        kt = kt_pool.tile([P, OC, T], QK_DT, name="kt", tag="kt")
        for sc in range(T // CW):
            kT_in = kT_chunks.pop()
            for ic in range(OC):
                ps = mm_ps.tile([P, TCH], F32, name="pjps", tag="mmps")
                for jc in range(OC):
                    _mm(nc, ps[:, :CW], gT[:, jc, ic * P:(ic + 1) * P],
                        kT_in[:, jc, :], QK_DT,
                        start=(jc == 0), stop=(jc == OC - 1))
                nc.vector.tensor_copy(
                    kt[:, ic, sc * CW:(sc + 1) * CW], ps[:, :CW])
            if sc + 1 < T // CW:
                kT_chunks.append(in_transpose_chunk(k_d, (sc + 1) * CW))

        _mark(nc, 'phaseWv')
        wvT = load_wT(wv_d, VP_DT)

        # ---- value loads first: DMA + bf16 cast run behind the other phases
        _mark(nc, 'phaseV')
        val = val_pool.tile([P, T // P, D], AV_DT)
        for tb in range(T // P):
            stg = stage.tile([P, D], F32, name="stage", tag="stage")
            dma(stg[:], v_d.ap()[tb * P:(tb + 1) * P, :])
            nc.gpsimd.tensor_copy(val[:, tb, :], stg[:])

        # ---- strip loop: 3-stage software pipeline ----
        state = {}

        def pass_a(si):
            """scores chunks + row max for strip si (PE + DVE)."""
            _mark(nc, f'strip{si}')
            s0 = si * P
            nch = (s0 + P + TCH - 1) // TCH   # t-chunks covering [0, s0+128)
            stg = stage.tile([P, D], F32, name="stage", tag="stage")
            dma(stg[:], q_d.ap()[s0:s0 + P, :])
            qts = qts_pool.tile([P, OC, P], QK_DT, name="qts")
            for g in range(0, OC, 4):
                ps = mm_ps.tile([P, TCH], F32, name="tps", tag="mmps")
                for c in range(4):
                    nc.tensor.transpose(
                        ps[:, c * P:(c + 1) * P],
                        stg[:, (g + c) * P:(g + c + 1) * P], ident[:])
                nc.vector.tensor_copy(
                    qts[:, g:g + 4, :],
                    ps.rearrange("p (c s) -> p c s", c=4))

            scores = sc_pool.tile([P, S], F32, name="scores")
            # diag chunk first so its (mask + max) tail overlaps the other
            # chunks' matmuls; per-chunk maxes keep the reduce off the
            # critical path
            cmaxes = []
            for c in [nch - 1] + list(range(nch - 1)):
                ps = mm_ps.tile([P, TCH], F32, name="scps", tag="mmps")
                for oc in range(OC):
                    _mm(nc, ps[:], qts[:, oc, :],
                        kt[:, oc, c * TCH:(c + 1) * TCH], QK_DT,
                        start=(oc == 0), stop=(oc == OC - 1))
                dst = scores[:, c * TCH:(c + 1) * TCH]
                if c < nch - 1:
                    nc.vector.tensor_copy(dst, ps[:])
                else:
                    nc.vector.tensor_add(dst, ps[:], diagmask[:, si % 4, :])
                cmax = st_pool.tile([P, 1], F32, name="cmax", tag="st")
                nc.vector.reduce_max(cmax[:], dst, axis=mybir.AxisListType.X)
                cmaxes.append(cmax)
            while len(cmaxes) > 1:
                nxt = []
                for a, b in zip(cmaxes[::2], cmaxes[1::2]):
                    m = st_pool.tile([P, 1], F32, name="cmax", tag="st")
                    nc.vector.tensor_max(m[:], a[:], b[:])
                    nxt.append(m)
                if len(cmaxes) % 2:
                    nxt.append(cmaxes[-1])
                cmaxes = nxt
            negm = st_pool.tile([P, 1], F32, name="negm", tag="st")
            nc.vector.tensor_scalar_mul(negm[:], cmaxes[0][:], -SCALE)
            state[si] = (scores, negm)

        def pass_b1(si):
            """exp + transpose to attnT + attn@value for strip si."""
            nch = (si * P + P + TCH - 1) // TCH
            ntb = si + 1                      # 128-wide t-blocks in play
            scores, negm = state.pop(si)

            attnT = at_pool.tile([P, NSTRIP, P], AV_DT, name="attnT")
            partials = []
            for c in range(nch):
                expc = exp_pool.tile([P, TCH], AV_DT, name="expc")
                part = st_pool.tile([P, 1], F32, name="part", tag="st")
                nc.scalar.activation(expc[:], scores[:, c * TCH:(c + 1) * TCH],
                                     mybir.ActivationFunctionType.Exp,
                                     bias=negm[:], scale=SCALE,
                                     accum_out=part[:])
                partials.append(part)
                nblk = min(4, ntb - 4 * c)    # skip all-zero blocks past diag
                ps = mm_ps.tile([P, TCH], AV_DT, name="tps2", tag="mmps")
                for g in range(nblk):
                    nc.tensor.transpose(ps[:, g * P:(g + 1) * P],
                                        expc[:, g * P:(g + 1) * P],
                                        ident_bf[:])
                nc.vector.tensor_copy(
                    attnT[:, 4 * c:4 * c + nblk, :],
                    ps[:, :nblk * P].rearrange("p (c s) -> p c s", c=nblk))
            rowsum = st_pool.tile([P, 1], F32, name="rowsum", tag="st")
            if len(partials) == 1:
                nc.vector.tensor_copy(rowsum[:], partials[0][:])
            else:
                nc.vector.tensor_add(rowsum[:], partials[0][:], partials[1][:])
                for part in partials[2:]:
                    nc.vector.tensor_add(rowsum[:], rowsum[:], part[:])

            # AV: ctx[s, i] = sum_t attn[s, t] * value[t, i]
            cps = ctx_ps.tile([P, 2, TCH], F32, name="ctxps")
            for tb in range(ntb):
                for ih in range(2):
                    nc.tensor.matmul(cps[:, ih, :],
                                     attnT[:, tb, :],
                                     val[:, tb, ih * TCH:(ih + 1) * TCH],
                                     start=(tb == 0), stop=(tb == ntb - 1))
            ctxn = ctxn_pool.tile([P, 2, TCH], F32, name="ctxn")
            nc.vector.tensor_copy(ctxn[:], cps[:])
            state[(si, 'b2')] = (ctxn, rowsum)

        def pass_b2(si):
            """transpose ctx -> ctxT, V-projection, normalize, store."""
            s0 = si * P
            ctxn, rowsum = state.pop((si, 'b2'))
            ctxsb = ctx_pool.tile([P, OC, P], VP_DT, name="ctxsb")
            for g in range(2):
                tp = mm_ps.tile([P, TCH], F32, name="ctp", tag="mmps")
                for c in range(4):
                    nc.tensor.transpose(
                        tp[:, c * P:(c + 1) * P],
                        ctxn[:, g, c * P:(c + 1) * P], ident[:])
                nc.vector.tensor_copy(
                    ctxsb[:, 4 * g:4 * g + 4, :],
                    tp.rearrange("p (c s) -> p c s", c=4))

            recip = st_pool.tile([P, 1], F32, name="recip", tag="st")
            nc.vector.reciprocal(recip[:], rowsum[:])
            for dc in range(2):
                ps = mm_ps.tile([P, TCH], F32, name="vops", tag="mmps")
                for ic in range(OC):
                    _mm(nc, ps[:], ctxsb[:, ic, :],
                        wvT[:, ic, dc * TCH:(dc + 1) * TCH], VP_DT,
                        start=(ic == 0), stop=(ic == OC - 1))
                ob = ob_pool.tile([P, TCH], F32, name="ob")
                nc.scalar.activation(ob[:], ps[:],
                                     mybir.ActivationFunctionType.Copy,
                                     scale=recip[:])
                dma(
                    out_d.ap()[s0:s0 + P, dc * TCH:(dc + 1) * TCH], ob[:])

        pass_a(0)
        pass_a(1)
        pass_b1(0)
        for si in range(NSTRIP):
            if si + 2 < NSTRIP:
                pass_a(si + 2)
            if si + 1 < NSTRIP:
                pass_b1(si + 1)
            pass_b2(si)

    _mark(nc, 'end')
    nc.finalize()
    return nc


_NC_CACHE = None


def kernel(**inputs):
    global _NC_CACHE
    if _NC_CACHE is None:
        _NC_CACHE = build_nc()
    nc = _NC_CACHE
    query = np.ascontiguousarray(inputs["query"], dtype=np.float32)
    key = np.ascontiguousarray(inputs["key"], dtype=np.float32)
    value = np.ascontiguousarray(inputs["value"], dtype=np.float32)
    Wq = np.ascontiguousarray(inputs["Wq"], dtype=np.float32)
    Wk = np.ascontiguousarray(inputs["Wk"], dtype=np.float32)
    Wv = np.ascontiguousarray(inputs["Wv"], dtype=np.float32)
    in_maps = [
        {"query": query[i], "key": key[i], "value": value[i],
         "Wq": Wq, "Wk": Wk, "Wv": Wv}
        for i in range(N)
    ]
    res = run_bass_kernel_spmd(nc, in_maps, core_ids=list(range(N)))
    return np.stack([res.results[i]["out"] for i in range(N)], axis=0)


# revision 20
# speedup vs baseline: 1.0318x; 1.0318x over previous
"""Causal attention layer on 8 TRN2 NeuronCores, data-parallel over batch.

Per-core problem (batch element n = core id):
    q = query @ Wq.T ; k = key @ Wk.T              (f32r matmuls)
    scores[s,t] = q[s]·k[t]  for t <= s            (f32r)
    attn = softmax(32 * scores)  (the +1 additive mask cancels in softmax;
                                  -inf masking == skipping t > s)
    ctx[s,i] = sum_t attn[s,t] * value[t,i]        (bf16)
    out[s,:] = (ctx @ Wv.T) / rowsum               (f32r, normalization folded)

Layouts: qT/kT are built as [D, S] via PE transposes of the inputs and
weights so every matmul contracts along partitions without DMA transposes.
The strip loop is a 3-stage software pipeline (A: scores+max, B1: exp+AV,
B2: ctx-transpose+V-proj) so the PE always has matmuls queued while the
serial softmax chain runs on DVE/ACT.
"""
import numpy as np
from contextlib import ExitStack

import concourse.bass as bass
import concourse.tile as tile
from concourse import bacc, mybir
from concourse.bass_utils import run_bass_kernel_spmd
from concourse.masks import make_identity

F32 = mybir.dt.float32
F32R = mybir.dt.float32r
BF16 = mybir.dt.bfloat16

N, S, T, D = 8, 2048, 2048, 1024
P = 128
NSTRIP = S // P          # 16 query strips
TCH = 512                # t-chunk for score matmuls
CW = 512                 # projection chunk width
OC = D // P              # 8 chunks of the projection/feature dim
SCALE = float(np.sqrt(np.float32(D)))  # 32.0
NEG = -1.0e30

# dtype knobs (QK path needs >= f32r precision; see noise_sim.py)
QK_DT = F32R             # q/k projections + scores matmuls
AV_DT = BF16             # attn weights + value contraction
VP_DT = F32R             # final (attn@value) @ Wv.T projection

PHASE_MARKS = []


def _mark(nc, label):
    PHASE_MARKS.append((label, nc.next_id()))


def _mm(nc, out, lhsT, rhs, dt, **kw):
    nc.tensor.matmul(out, lhsT.bitcast(dt), rhs.bitcast(dt), **kw)


def build_nc():
    PHASE_MARKS.clear()
    nc = bacc.Bacc("TRN2", target_bir_lowering=False, debug=False,
                   enable_asserts=False)
    _dma_rr = [0]

    def dma(out, in_):
        # alternate between the two HWDGE queues (sync/SP and scalar/ACT)
        _dma_rr[0] ^= 1
        eng = nc.sync if _dma_rr[0] else nc.scalar
        return eng.dma_start(out, in_)
    q_d = nc.dram_tensor("query", [S, D], F32, kind="ExternalInput")
    k_d = nc.dram_tensor("key", [T, D], F32, kind="ExternalInput")
    v_d = nc.dram_tensor("value", [T, D], F32, kind="ExternalInput")
    wq_d = nc.dram_tensor("Wq", [D, D], F32, kind="ExternalInput")
    wk_d = nc.dram_tensor("Wk", [D, D], F32, kind="ExternalInput")
    wv_d = nc.dram_tensor("Wv", [D, D], F32, kind="ExternalInput")
    out_d = nc.dram_tensor("out", [S, D], F32, kind="ExternalOutput")

    with tile.TileContext(nc) as tc, ExitStack() as ctx:
        const = ctx.enter_context(tc.tile_pool(name="const", bufs=1))
        wt_pool = ctx.enter_context(tc.tile_pool(name="wt", bufs=1))
        kt_pool = ctx.enter_context(tc.tile_pool(name="kt", bufs=1))
        val_pool = ctx.enter_context(tc.tile_pool(name="val", bufs=1))
        stage = ctx.enter_context(tc.tile_pool(name="stage", bufs=3))
        int_pool = ctx.enter_context(tc.tile_pool(name="inT", bufs=1))
        qts_pool = ctx.enter_context(tc.tile_pool(name="qts", bufs=2))
        sc_pool = ctx.enter_context(tc.tile_pool(name="scores", bufs=2))
        exp_pool = ctx.enter_context(tc.tile_pool(name="exp", bufs=2))
        at_pool = ctx.enter_context(tc.tile_pool(name="attnT", bufs=1))
        ctxn_pool = ctx.enter_context(tc.tile_pool(name="ctxn", bufs=2))
        ctx_pool = ctx.enter_context(tc.tile_pool(name="ctxsb", bufs=1))
        ob_pool = ctx.enter_context(tc.tile_pool(name="outb", bufs=1))
        st_pool = ctx.enter_context(tc.tile_pool(name="stats", bufs=24))
        mm_ps = ctx.enter_context(tc.tile_pool(name="mmps", bufs=4, space="PSUM"))
        ctx_ps = ctx.enter_context(tc.tile_pool(name="ctxps", bufs=2, space="PSUM"))

        ident = const.tile([P, P], F32)
        make_identity(nc, ident)
        ident_bf = const.tile([P, P], BF16)
        nc.vector.tensor_copy(ident_bf[:], ident[:])

        # Additive diag masks for the 4 possible strip positions inside a
        # 512-wide t-chunk: mask[j][x, y] = 0 if y <= j*128 + x else -1e30
        # (bf16 is exact for both values)
        diagmask = const.tile([P, 4, TCH], BF16)
        nc.gpsimd.memset(diagmask[:], 0.0)
        for j in range(4):
            nc.gpsimd.affine_select(
                out=diagmask[:, j, :], in_=diagmask[:, j, :],
                compare_op=mybir.AluOpType.is_ge, fill=NEG,
                base=j * P, channel_multiplier=1, pattern=[[-1, TCH]])

        def load_wT(w_dram, dt):
            """[D,D] weight [o,i] -> SBUF [128(i_loc), OC(i_chunk), D(o)]."""
            wt = wt_pool.tile([P, OC, D], dt, name="wT", tag="wT")
            for r in range(OC):              # o-strip
                stg = stage.tile([P, D], F32, name="stage", tag="stage")
                dma(stg[:], w_dram.ap()[r * P:(r + 1) * P, :])
                for g in range(0, OC, 4):
                    ps = mm_ps.tile([P, TCH], F32, name="tps", tag="mmps")
                    for c in range(4):
                        nc.tensor.transpose(
                            ps[:, c * P:(c + 1) * P],
                            stg[:, (g + c) * P:(g + c + 1) * P], ident[:])
                    nc.vector.tensor_copy(
                        wt[:, g:g + 4, r * P:(r + 1) * P],
                        ps.rearrange("p (c s) -> p c s", c=4))
            return wt

        def in_transpose_chunk(x_dram, c0):
            """x[c0:c0+CW, :D] -> xT tile [128(i_loc), OC, CW]."""
            xt = int_pool.tile([P, OC, CW], QK_DT, name="inT", tag="inT")
            for sl in range(CW // P):
                stg = stage.tile([P, D], F32, name="stage", tag="stage")
                dma(
                    stg[:], x_dram.ap()[c0 + sl * P:c0 + (sl + 1) * P, :])
                for g in range(0, OC, 4):
                    ps = mm_ps.tile([P, TCH], F32, name="tps", tag="mmps")
                    for c in range(4):
                        nc.tensor.transpose(
                            ps[:, c * P:(c + 1) * P],
                            stg[:, (g + c) * P:(g + c + 1) * P], ident[:])
                    nc.vector.tensor_copy(
                        xt[:, g:g + 4, sl * P:(sl + 1) * P],
                        ps.rearrange("p (c s) -> p c s", c=4))
            return xt

        # ---- phase G: gT[j, i] = (Wq^T @ Wk)^T = Wk^T @ Wq ----
        # Both weights are consumed in their NATURAL [o, .] layout, so no
        # weight transposes are needed; Wq's projection of the query side
        # is folded into the key side via M = G @ key^T below.
        _mark(nc, 'phaseG')
        wqwk = kt_pool.tile([P, 16, D], QK_DT, name="wqwk", tag="kt")
        for r in range(OC):
            stg = stage.tile([P, D], F32, name="stage", tag="stage")
            dma(stg[:], wq_d.ap()[r * P:(r + 1) * P, :])
            nc.vector.tensor_copy(wqwk[:, r, :], stg[:])
            stg2 = stage.tile([P, D], F32, name="stage", tag="stage")
            dma(stg2[:], wk_d.ap()[r * P:(r + 1) * P, :])
            nc.vector.tensor_copy(wqwk[:, OC + r, :], stg2[:])
        kT_chunks = [in_transpose_chunk(k_d, 0)]
        gT = wt_pool.tile([P, OC, D], QK_DT, name="wT", tag="wT")
        # o-strip-streamed accumulation, 4 jc-accumulators at a time in the
        # (preamble-idle) ctx_ps banks; matmuls start as soon as the first
        # Wq/Wk strips land and the group copies overlap the next group
        for ih in range(2):
            for jcg in range(2):
                acc_b = ctx_ps.tile([P, 2, TCH], F32, name="ctxps",
                                    tag="ctxps")
                acc_c = ctx_ps.tile([P, 2, TCH], F32, name="ctxps",
                                    tag="ctxps")
                accs = [acc_b[:, 0, :], acc_b[:, 1, :],
                        acc_c[:, 0, :], acc_c[:, 1, :]]
                for oc in range(OC):
                    for j in range(4):
                        jc = jcg * 4 + j
                        _mm(nc, accs[j],
                            wqwk[:, OC + oc, jc * P:(jc + 1) * P],
                            wqwk[:, oc, ih * TCH:(ih + 1) * TCH], QK_DT,
                            start=(oc == 0), stop=(oc == OC - 1))
                for j in range(4):
                    jc = jcg * 4 + j
                    nc.vector.tensor_copy(
                        gT[:, jc, ih * TCH:(ih + 1) * TCH], accs[j])

        # ---- phase M: M = G @ key^T -> resident SBUF [128, OC, T] ----
        # (reuses the wqwk slot; M plays the role kT did: scores = query @ M)
        _mark(nc, 'phaseM')
        kt = kt_pool.tile([P, OC, T], QK_DT, name="kt", tag="kt")
        for sc in range(T // CW):
            kT_in = kT_chunks.pop()
            for ic in range(OC):
                ps = mm_ps.tile([P, TCH], F32, name="pjps", tag="mmps")
                for jc in range(OC):
                    _mm(nc, ps[:, :CW], gT[:, jc, ic * P:(ic + 1) * P],
                        kT_in[:, jc, :], QK_DT,
                        start=(jc == 0), stop=(jc == OC - 1))
                nc.vector.tensor_copy(
                    kt[:, ic, sc * CW:(sc + 1) * CW], ps[:, :CW])
            if sc + 1 < T // CW:
                kT_chunks.append(in_transpose_chunk(k_d, (sc + 1) * CW))

        _mark(nc, 'phaseWv')
        wvT = load_wT(wv_d, VP_DT)

        # ---- value loads first: DMA + bf16 cast run behind the other phases
        _mark(nc, 'phaseV')
        val = val_pool.tile([P, T // P, D], AV_DT)
        for tb in range(T // P):
            stg = stage.tile([P, D], F32, name="stage", tag="stage")
            dma(stg[:], v_d.ap()[tb * P:(tb + 1) * P, :])
            nc.gpsimd.tensor_copy(val[:, tb, :], stg[:])

        # ---- strip loop: 3-stage software pipeline ----
        state = {}

        def pass_a(si):
            """scores chunks + row max for strip si (PE + DVE)."""
            _mark(nc, f'strip{si}')
            s0 = si * P
            nch = (s0 + P + TCH - 1) // TCH   # t-chunks covering [0, s0+128)
            stg = stage.tile([P, D], F32, name="stage", tag="stage")
            dma(stg[:], q_d.ap()[s0:s0 + P, :])
            qts = qts_pool.tile([P, OC, P], QK_DT, name="qts")
            for g in range(0, OC, 4):
                ps = mm_ps.tile([P, TCH], F32, name="tps", tag="mmps")
                for c in range(4):
                    nc.tensor.transpose(
                        ps[:, c * P:(c + 1) * P],
                        stg[:, (g + c) * P:(g + c + 1) * P], ident[:])
                nc.vector.tensor_copy(
                    qts[:, g:g + 4, :],
                    ps.rearrange("p (c s) -> p c s", c=4))

            scores = sc_pool.tile([P, S], F32, name="scores")
            # diag chunk first so its (mask + max) tail overlaps the other
            # chunks' matmuls; per-chunk maxes keep the reduce off the
            # critical path
            cmaxes = []
            for c in [nch - 1] + list(range(nch - 1)):
                ps = mm_ps.tile([P, TCH], F32, name="scps", tag="mmps")
                for oc in range(OC):
                    _mm(nc, ps[:], qts[:, oc, :],
                        kt[:, oc, c * TCH:(c + 1) * TCH], QK_DT,
                        start=(oc == 0), stop=(oc == OC - 1))
                dst = scores[:, c * TCH:(c + 1) * TCH]
                if c < nch - 1:
                    nc.vector.tensor_copy(dst, ps[:])
                else:
                    nc.vector.tensor_add(dst, ps[:], diagmask[:, si % 4, :])
                cmax = st_pool.tile([P, 1], F32, name="cmax", tag="st")
                nc.vector.reduce_max(cmax[:], dst, axis=mybir.AxisListType.X)
                cmaxes.append(cmax)
            while len(cmaxes) > 1:
                nxt = []
                for a, b in zip(cmaxes[::2], cmaxes[1::2]):
                    m = st_pool.tile([P, 1], F32, name="cmax", tag="st")
                    nc.vector.tensor_max(m[:], a[:], b[:])
                    nxt.append(m)
                if len(cmaxes) % 2:
                    nxt.append(cmaxes[-1])
                cmaxes = nxt
            negm = st_pool.tile([P, 1], F32, name="negm", tag="st")
            nc.vector.tensor_scalar_mul(negm[:], cmaxes[0][:], -SCALE)
            state[si] = (scores, negm)

        def pass_b1(si):
            """exp + transpose to attnT + attn@value for strip si."""
            nch = (si * P + P + TCH - 1) // TCH
            ntb = si + 1                      # 128-wide t-blocks in play
            scores, negm = state.pop(si)

            attnT = at_pool.tile([P, NSTRIP, P], AV_DT, name="attnT")
            partials = []
            for c in range(nch):
                expc = exp_pool.tile([P, TCH], AV_DT, name="expc")
                part = st_pool.tile([P, 1], F32, name="part", tag="st")
                nc.scalar.activation(expc[:], scores[:, c * TCH:(c + 1) * TCH],
                                     mybir.ActivationFunctionType.Exp,
                                     bias=negm[:], scale=SCALE,
                                     accum_out=part[:])
                partials.append(part)
                nblk = min(4, ntb - 4 * c)    # skip all-zero blocks past diag
                ps = mm_ps.tile([P, TCH], AV_DT, name="tps2", tag="mmps")
                for g in range(nblk):
                    nc.tensor.transpose(ps[:, g * P:(g + 1) * P],
                                        expc[:, g * P:(g + 1) * P],
                                        ident_bf[:])
                nc.vector.tensor_copy(
                    attnT[:, 4 * c:4 * c + nblk, :],
                    ps[:, :nblk * P].rearrange("p (c s) -> p c s", c=nblk))
            rowsum = st_pool.tile([P, 1], F32, name="rowsum", tag="st")
            if len(partials) == 1:
                nc.vector.tensor_copy(rowsum[:], partials[0][:])
            else:
                nc.vector.tensor_add(rowsum[:], partials[0][:], partials[1][:])
                for part in partials[2:]:
                    nc.vector.tensor_add(rowsum[:], rowsum[:], part[:])

            # AV: ctx[s, i] = sum_t attn[s, t] * value[t, i]
            cps = ctx_ps.tile([P, 2, TCH], F32, name="ctxps")
            for tb in range(ntb):
                for ih in range(2):
                    nc.tensor.matmul(cps[:, ih, :],
                                     attnT[:, tb, :],
                                     val[:, tb, ih * TCH:(ih + 1) * TCH],
                                     start=(tb == 0), stop=(tb == ntb - 1))
            ctxn = ctxn_pool.tile([P, 2, TCH], F32, name="ctxn")
            nc.vector.tensor_copy(ctxn[:], cps[:])
            state[(si, 'b2')] = (ctxn, rowsum)

        def pass_b2(si):
            """transpose ctx -> ctxT, V-projection, normalize, store."""
            s0 = si * P
            ctxn, rowsum = state.pop((si, 'b2'))
            ctxsb = ctx_pool.tile([P, OC, P], VP_DT, name="ctxsb")
            for g in range(2):
                tp = mm_ps.tile([P, TCH], F32, name="ctp", tag="mmps")
                for c in range(4):
                    nc.tensor.transpose(
                        tp[:, c * P:(c + 1) * P],
                        ctxn[:, g, c * P:(c + 1) * P], ident[:])
                nc.vector.tensor_copy(
                    ctxsb[:, 4 * g:4 * g + 4, :],
                    tp.rearrange("p (c s) -> p c s", c=4))

            recip = st_pool.tile([P, 1], F32, name="recip", tag="st")
            nc.vector.reciprocal(recip[:], rowsum[:])
            for dc in range(2):
                ps = mm_ps.tile([P, TCH], F32, name="vops", tag="mmps")
                for ic in range(OC):
                    _mm(nc, ps[:], ctxsb[:, ic, :],
                        wvT[:, ic, dc * TCH:(dc + 1) * TCH], VP_DT,
                        start=(ic == 0), stop=(ic == OC - 1))
                ob = ob_pool.tile([P, TCH], F32, name="ob")
                nc.scalar.activation(ob[:], ps[:],
                                     mybir.ActivationFunctionType.Copy,
                                     scale=recip[:])
                dma(
                    out_d.ap()[s0:s0 + P, dc * TCH:(dc + 1) * TCH], ob[:])

        pass_a(0)
        pass_a(1)
        pass_b1(0)
        for si in range(NSTRIP):
            if si + 2 < NSTRIP:
                pass_a(si + 2)
            if si + 1 < NSTRIP:
                pass_b1(si + 1)
            pass_b2(si)

    _mark(nc, 'end')
    nc.finalize()
    return nc


_NC_CACHE = None


def kernel(**inputs):
    global _NC_CACHE
    if _NC_CACHE is None:
        _NC_CACHE = build_nc()
    nc = _NC_CACHE
    query = np.ascontiguousarray(inputs["query"], dtype=np.float32)
    key = np.ascontiguousarray(inputs["key"], dtype=np.float32)
    value = np.ascontiguousarray(inputs["value"], dtype=np.float32)
    Wq = np.ascontiguousarray(inputs["Wq"], dtype=np.float32)
    Wk = np.ascontiguousarray(inputs["Wk"], dtype=np.float32)
    Wv = np.ascontiguousarray(inputs["Wv"], dtype=np.float32)
    in_maps = [
        {"query": query[i], "key": key[i], "value": value[i],
         "Wq": Wq, "Wk": Wk, "Wv": Wv}
        for i in range(N)
    ]
    res = run_bass_kernel_spmd(nc, in_maps, core_ids=list(range(N)))
    return np.stack([res.results[i]["out"] for i in range(N)], axis=0)


# revision 21
# speedup vs baseline: 1.0519x; 1.0195x over previous
"""Causal attention layer on 8 TRN2 NeuronCores, data-parallel over batch.

Per-core problem (batch element n = core id):
    q = query @ Wq.T ; k = key @ Wk.T              (f32r matmuls)
    scores[s,t] = q[s]·k[t]  for t <= s            (f32r)
    attn = softmax(32 * scores)  (the +1 additive mask cancels in softmax;
                                  -inf masking == skipping t > s)
    ctx[s,i] = sum_t attn[s,t] * value[t,i]        (bf16)
    out[s,:] = (ctx @ Wv.T) / rowsum               (f32r, normalization folded)

Layouts: qT/kT are built as [D, S] via PE transposes of the inputs and
weights so every matmul contracts along partitions without DMA transposes.
The strip loop is a 3-stage software pipeline (A: scores+max, B1: exp+AV,
B2: ctx-transpose+V-proj) so the PE always has matmuls queued while the
serial softmax chain runs on DVE/ACT.
"""
import numpy as np
from contextlib import ExitStack

import concourse.bass as bass
import concourse.tile as tile
from concourse import bacc, mybir
from concourse.bass_utils import run_bass_kernel_spmd
from concourse.masks import make_identity

F32 = mybir.dt.float32
F32R = mybir.dt.float32r
BF16 = mybir.dt.bfloat16

N, S, T, D = 8, 2048, 2048, 1024
P = 128
NSTRIP = S // P          # 16 query strips
TCH = 512                # t-chunk for score matmuls
CW = 512                 # projection chunk width
OC = D // P              # 8 chunks of the projection/feature dim
SCALE = float(np.sqrt(np.float32(D)))  # 32.0
NEG = -1.0e30

# dtype knobs (QK path needs >= f32r precision; see noise_sim.py)
QK_DT = F32R             # q/k projections + scores matmuls
AV_DT = BF16             # attn weights + value contraction
VP_DT = F32R             # final (attn@value) @ Wv.T projection

PHASE_MARKS = []


def _mark(nc, label):
    PHASE_MARKS.append((label, nc.next_id()))


def _mm(nc, out, lhsT, rhs, dt, **kw):
    nc.tensor.matmul(out, lhsT.bitcast(dt), rhs.bitcast(dt), **kw)


def build_nc():
    PHASE_MARKS.clear()
    nc = bacc.Bacc("TRN2", target_bir_lowering=False, debug=False,
                   enable_asserts=False)
    _dma_rr = [0]

    def dma(out, in_):
        # alternate between the two HWDGE queues (sync/SP and scalar/ACT)
        _dma_rr[0] ^= 1
        eng = nc.sync if _dma_rr[0] else nc.scalar
        return eng.dma_start(out, in_)
    q_d = nc.dram_tensor("query", [S, D], F32, kind="ExternalInput")
    k_d = nc.dram_tensor("key", [T, D], F32, kind="ExternalInput")
    v_d = nc.dram_tensor("value", [T, D], F32, kind="ExternalInput")
    wq_d = nc.dram_tensor("Wq", [D, D], F32, kind="ExternalInput")
    wk_d = nc.dram_tensor("Wk", [D, D], F32, kind="ExternalInput")
    wv_d = nc.dram_tensor("Wv", [D, D], F32, kind="ExternalInput")
    out_d = nc.dram_tensor("out", [S, D], F32, kind="ExternalOutput")

    with tile.TileContext(nc) as tc, ExitStack() as ctx:
        const = ctx.enter_context(tc.tile_pool(name="const", bufs=1))
        wt_pool = ctx.enter_context(tc.tile_pool(name="wt", bufs=1))
        kt_pool = ctx.enter_context(tc.tile_pool(name="kt", bufs=1))
        val_pool = ctx.enter_context(tc.tile_pool(name="val", bufs=1))
        stage = ctx.enter_context(tc.tile_pool(name="stage", bufs=2))
        vstage = ctx.enter_context(tc.tile_pool(name="vstage", bufs=1))
        int_pool = ctx.enter_context(tc.tile_pool(name="inT", bufs=1))
        qts_pool = ctx.enter_context(tc.tile_pool(name="qts", bufs=2))
        sc_pool = ctx.enter_context(tc.tile_pool(name="scores", bufs=2))
        exp_pool = ctx.enter_context(tc.tile_pool(name="exp", bufs=2))
        at_pool = ctx.enter_context(tc.tile_pool(name="attnT", bufs=1))
        ctxn_pool = ctx.enter_context(tc.tile_pool(name="ctxn", bufs=2))
        ctx_pool = ctx.enter_context(tc.tile_pool(name="ctxsb", bufs=1))
        ob_pool = ctx.enter_context(tc.tile_pool(name="outb", bufs=1))
        st_pool = ctx.enter_context(tc.tile_pool(name="stats", bufs=24))
        mm_ps = ctx.enter_context(tc.tile_pool(name="mmps", bufs=4, space="PSUM"))
        ctx_ps = ctx.enter_context(tc.tile_pool(name="ctxps", bufs=2, space="PSUM"))

        ident = const.tile([P, P], F32)
        make_identity(nc, ident)
        ident_bf = const.tile([P, P], BF16)
        nc.vector.tensor_copy(ident_bf[:], ident[:])

        # Additive diag masks for the 4 possible strip positions inside a
        # 512-wide t-chunk: mask[j][x, y] = 0 if y <= j*128 + x else -1e30
        # (bf16 is exact for both values)
        diagmask = const.tile([P, 4, TCH], BF16)
        nc.gpsimd.memset(diagmask[:], 0.0)
        for j in range(4):
            nc.gpsimd.affine_select(
                out=diagmask[:, j, :], in_=diagmask[:, j, :],
                compare_op=mybir.AluOpType.is_ge, fill=NEG,
                base=j * P, channel_multiplier=1, pattern=[[-1, TCH]])

        def load_wT(w_dram, dt):
            """[D,D] weight [o,i] -> SBUF [128(i_loc), OC(i_chunk), D(o)]."""
            wt = wt_pool.tile([P, OC, D], dt, name="wT", tag="wT")
            for r in range(OC):              # o-strip
                stg = stage.tile([P, D], F32, name="stage", tag="stage")
                dma(stg[:], w_dram.ap()[r * P:(r + 1) * P, :])
                for g in range(0, OC, 4):
                    ps = mm_ps.tile([P, TCH], F32, name="tps", tag="mmps")
                    for c in range(4):
                        nc.tensor.transpose(
                            ps[:, c * P:(c + 1) * P],
                            stg[:, (g + c) * P:(g + c + 1) * P], ident[:])
                    nc.vector.tensor_copy(
                        wt[:, g:g + 4, r * P:(r + 1) * P],
                        ps.rearrange("p (c s) -> p c s", c=4))
            return wt

        def in_transpose_chunk(x_dram, c0):
            """x[c0:c0+CW, :D] -> xT tile [128(i_loc), OC, CW]."""
            xt = int_pool.tile([P, OC, CW], QK_DT, name="inT", tag="inT")
            for sl in range(CW // P):
                stg = stage.tile([P, D], F32, name="stage", tag="stage")
                dma(
                    stg[:], x_dram.ap()[c0 + sl * P:c0 + (sl + 1) * P, :])
                for g in range(0, OC, 4):
                    ps = mm_ps.tile([P, TCH], F32, name="tps", tag="mmps")
                    for c in range(4):
                        nc.tensor.transpose(
                            ps[:, c * P:(c + 1) * P],
                            stg[:, (g + c) * P:(g + c + 1) * P], ident[:])
                    nc.vector.tensor_copy(
                        xt[:, g:g + 4, sl * P:(sl + 1) * P],
                        ps.rearrange("p (c s) -> p c s", c=4))
            return xt

        # ---- phase G: gT[j, i] = (Wq^T @ Wk)^T = Wk^T @ Wq ----
        # Both weights are consumed in their NATURAL [o, .] layout, so no
        # weight transposes are needed; Wq's projection of the query side
        # is folded into the key side via M = G @ key^T below.
        _mark(nc, 'phaseG')
        wqwk = kt_pool.tile([P, 16, D], QK_DT, name="wqwk", tag="kt")
        for r in range(OC):
            stg = stage.tile([P, D], F32, name="stage", tag="stage")
            dma(stg[:], wq_d.ap()[r * P:(r + 1) * P, :])
            nc.vector.tensor_copy(wqwk[:, r, :], stg[:])
            stg2 = stage.tile([P, D], F32, name="stage", tag="stage")
            dma(stg2[:], wk_d.ap()[r * P:(r + 1) * P, :])
            nc.vector.tensor_copy(wqwk[:, OC + r, :], stg2[:])
        kT_chunks = [in_transpose_chunk(k_d, 0)]
        gT = wt_pool.tile([P, OC, D], QK_DT, name="wT", tag="wT")
        # o-strip-streamed accumulation, 4 jc-accumulators at a time in the
        # (preamble-idle) ctx_ps banks; matmuls start as soon as the first
        # Wq/Wk strips land and the group copies overlap the next group
        for ih in range(2):
            for jcg in range(2):
                acc_b = ctx_ps.tile([P, 2, TCH], F32, name="ctxps",
                                    tag="ctxps")
                acc_c = ctx_ps.tile([P, 2, TCH], F32, name="ctxps",
                                    tag="ctxps")
                accs = [acc_b[:, 0, :], acc_b[:, 1, :],
                        acc_c[:, 0, :], acc_c[:, 1, :]]
                for oc in range(OC):
                    for j in range(4):
                        jc = jcg * 4 + j
                        _mm(nc, accs[j],
                            wqwk[:, OC + oc, jc * P:(jc + 1) * P],
                            wqwk[:, oc, ih * TCH:(ih + 1) * TCH], QK_DT,
                            start=(oc == 0), stop=(oc == OC - 1))
                for j in range(4):
                    jc = jcg * 4 + j
                    nc.vector.tensor_copy(
                        gT[:, jc, ih * TCH:(ih + 1) * TCH], accs[j])

        # ---- phase M: M = G @ key^T -> resident SBUF [128, OC, T] ----
        # (reuses the wqwk slot; M plays the role kT did: scores = query @ M)
        _mark(nc, 'phaseM')
        kt = kt_pool.tile([P, OC, T], QK_DT, name="kt", tag="kt")
        for sc in range(T // CW):
            kT_in = kT_chunks.pop()
            for ic in range(OC):
                ps = mm_ps.tile([P, TCH], F32, name="pjps", tag="mmps")
                for jc in range(OC):
                    _mm(nc, ps[:, :CW], gT[:, jc, ic * P:(ic + 1) * P],
                        kT_in[:, jc, :], QK_DT,
                        start=(jc == 0), stop=(jc == OC - 1))
                nc.vector.tensor_copy(
                    kt[:, ic, sc * CW:(sc + 1) * CW], ps[:, :CW])
            if sc + 1 < T // CW:
                kT_chunks.append(in_transpose_chunk(k_d, (sc + 1) * CW))

        _mark(nc, 'phaseWv')
        wvT = load_wT(wv_d, VP_DT)

        # ---- value loads first: DMA + bf16 cast run behind the other phases
        _mark(nc, 'phaseV')
        val = val_pool.tile([P, T // P, D], AV_DT)
        for tb in range(T // P):
            stg = vstage.tile([P, D], F32, name="vstage")
            dma(stg[:], v_d.ap()[tb * P:(tb + 1) * P, :])
            nc.gpsimd.tensor_copy(val[:, tb, :], stg[:])

        # ---- strip loop: 3-stage software pipeline ----
        state = {}

        def pass_a(si):
            """scores chunks + row max for strip si (PE + DVE)."""
            _mark(nc, f'strip{si}')
            s0 = si * P
            nch = (s0 + P + TCH - 1) // TCH   # t-chunks covering [0, s0+128)
            stg = stage.tile([P, D], F32, name="stage", tag="stage")
            dma(stg[:], q_d.ap()[s0:s0 + P, :])
            qts = qts_pool.tile([P, OC, P], QK_DT, name="qts")
            for g in range(0, OC, 4):
                ps = mm_ps.tile([P, TCH], F32, name="tps", tag="mmps")
                for c in range(4):
                    nc.tensor.transpose(
                        ps[:, c * P:(c + 1) * P],
                        stg[:, (g + c) * P:(g + c + 1) * P], ident[:])
                nc.vector.tensor_copy(
                    qts[:, g:g + 4, :],
                    ps.rearrange("p (c s) -> p c s", c=4))

            scores = sc_pool.tile([P, S], F32, name="scores")
            # diag chunk first so its (mask + max) tail overlaps the other
            # chunks' matmuls; per-chunk maxes keep the reduce off the
            # critical path
            cmaxes = []
            for c in [nch - 1] + list(range(nch - 1)):
                ps = mm_ps.tile([P, TCH], F32, name="scps", tag="mmps")
                for oc in range(OC):
                    _mm(nc, ps[:], qts[:, oc, :],
                        kt[:, oc, c * TCH:(c + 1) * TCH], QK_DT,
                        start=(oc == 0), stop=(oc == OC - 1))
                dst = scores[:, c * TCH:(c + 1) * TCH]
                if c < nch - 1:
                    nc.vector.tensor_copy(dst, ps[:])
                else:
                    nc.vector.tensor_add(dst, ps[:], diagmask[:, si % 4, :])
                cmax = st_pool.tile([P, 1], F32, name="cmax", tag="st")
                nc.vector.reduce_max(cmax[:], dst, axis=mybir.AxisListType.X)
                cmaxes.append(cmax)
            while len(cmaxes) > 1:
                nxt = []
                for a, b in zip(cmaxes[::2], cmaxes[1::2]):
                    m = st_pool.tile([P, 1], F32, name="cmax", tag="st")
                    nc.vector.tensor_max(m[:], a[:], b[:])
                    nxt.append(m)
                if len(cmaxes) % 2:
                    nxt.append(cmaxes[-1])
                cmaxes = nxt
            negm = st_pool.tile([P, 1], F32, name="negm", tag="st")
            nc.vector.tensor_scalar_mul(negm[:], cmaxes[0][:], -SCALE)
            state[si] = (scores, negm)

        def pass_b1(si):
            """exp + transpose to attnT + attn@value for strip si."""
            nch = (si * P + P + TCH - 1) // TCH
            ntb = si + 1                      # 128-wide t-blocks in play
            scores, negm = state.pop(si)

            attnT = at_pool.tile([P, NSTRIP, P], AV_DT, name="attnT")
            partials = []
            for c in range(nch):
                expc = exp_pool.tile([P, TCH], AV_DT, name="expc")
                part = st_pool.tile([P, 1], F32, name="part", tag="st")
                nc.scalar.activation(expc[:], scores[:, c * TCH:(c + 1) * TCH],
                                     mybir.ActivationFunctionType.Exp,
                                     bias=negm[:], scale=SCALE,
                                     accum_out=part[:])
                partials.append(part)
                nblk = min(4, ntb - 4 * c)    # skip all-zero blocks past diag
                ps = mm_ps.tile([P, TCH], AV_DT, name="tps2", tag="mmps")
                for g in range(nblk):
                    nc.tensor.transpose(ps[:, g * P:(g + 1) * P],
                                        expc[:, g * P:(g + 1) * P],
                                        ident_bf[:])
                nc.vector.tensor_copy(
                    attnT[:, 4 * c:4 * c + nblk, :],
                    ps[:, :nblk * P].rearrange("p (c s) -> p c s", c=nblk))
            rowsum = st_pool.tile([P, 1], F32, name="rowsum", tag="st")
            if len(partials) == 1:
                nc.vector.tensor_copy(rowsum[:], partials[0][:])
            else:
                nc.vector.tensor_add(rowsum[:], partials[0][:], partials[1][:])
                for part in partials[2:]:
                    nc.vector.tensor_add(rowsum[:], rowsum[:], part[:])

            # AV: ctx[s, i] = sum_t attn[s, t] * value[t, i]
            cps = ctx_ps.tile([P, 2, TCH], F32, name="ctxps")
            for tb in range(ntb):
                for ih in range(2):
                    nc.tensor.matmul(cps[:, ih, :],
                                     attnT[:, tb, :],
                                     val[:, tb, ih * TCH:(ih + 1) * TCH],
                                     start=(tb == 0), stop=(tb == ntb - 1))
            ctxn = ctxn_pool.tile([P, 2, TCH], F32, name="ctxn")
            nc.vector.tensor_copy(ctxn[:], cps[:])
            state[(si, 'b2')] = (ctxn, rowsum)

        def pass_b2(si):
            """transpose ctx -> ctxT, V-projection, normalize, store."""
            s0 = si * P
            ctxn, rowsum = state.pop((si, 'b2'))
            ctxsb = ctx_pool.tile([P, OC, P], VP_DT, name="ctxsb")
            for g in range(2):
                tp = mm_ps.tile([P, TCH], F32, name="ctp", tag="mmps")
                for c in range(4):
                    nc.tensor.transpose(
                        tp[:, c * P:(c + 1) * P],
                        ctxn[:, g, c * P:(c + 1) * P], ident[:])
                nc.vector.tensor_copy(
                    ctxsb[:, 4 * g:4 * g + 4, :],
                    tp.rearrange("p (c s) -> p c s", c=4))

            recip = st_pool.tile([P, 1], F32, name="recip", tag="st")
            nc.vector.reciprocal(recip[:], rowsum[:])
            for dc in range(2):
                ps = mm_ps.tile([P, TCH], F32, name="vops", tag="mmps")
                for ic in range(OC):
                    _mm(nc, ps[:], ctxsb[:, ic, :],
                        wvT[:, ic, dc * TCH:(dc + 1) * TCH], VP_DT,
                        start=(ic == 0), stop=(ic == OC - 1))
                ob = ob_pool.tile([P, TCH], F32, name="ob")
                nc.scalar.activation(ob[:], ps[:],
                                     mybir.ActivationFunctionType.Copy,
                                     scale=recip[:])
                dma(
                    out_d.ap()[s0:s0 + P, dc * TCH:(dc + 1) * TCH], ob[:])

        pass_a(0)
        pass_a(1)
        pass_b1(0)
        for si in range(NSTRIP):
            if si + 2 < NSTRIP:
                pass_a(si + 2)
            if si + 1 < NSTRIP:
                pass_b1(si + 1)
            pass_b2(si)

    _mark(nc, 'end')
    nc.finalize()
    return nc


_NC_CACHE = None


def kernel(**inputs):
    global _NC_CACHE
    if _NC_CACHE is None:
        _NC_CACHE = build_nc()
    nc = _NC_CACHE
    query = np.ascontiguousarray(inputs["query"], dtype=np.float32)
    key = np.ascontiguousarray(inputs["key"], dtype=np.float32)
    value = np.ascontiguousarray(inputs["value"], dtype=np.float32)
    Wq = np.ascontiguousarray(inputs["Wq"], dtype=np.float32)
    Wk = np.ascontiguousarray(inputs["Wk"], dtype=np.float32)
    Wv = np.ascontiguousarray(inputs["Wv"], dtype=np.float32)
    in_maps = [
        {"query": query[i], "key": key[i], "value": value[i],
         "Wq": Wq, "Wk": Wk, "Wv": Wv}
        for i in range(N)
    ]
    res = run_bass_kernel_spmd(nc, in_maps, core_ids=list(range(N)))
    return np.stack([res.results[i]["out"] for i in range(N)], axis=0)
